# revision 9
# baseline (speedup 1.0000x reference)
"""Trainium2 Bass kernel for nn_Attention_58798102283008.

Dense causal transformer attention block:
    qkv = x @ wqkv.T ; RoPE(q, k) ; causal softmax attention ; out @ wo.T
Shapes: x (2, 2048, 2048), wqkv (6144, 2048), wo (2048, 2048), f32.

Sharding over 8 NeuronCores: data parallel on batch (2) x tensor parallel on
heads (32 heads -> 4 groups of 8). Core c handles batch c//4, head group c%4.
Each core computes its heads' attention and a partial output projection
(wo column block); the host sums the 4 partials per batch (the unshard step
for this tensor-parallel decomposition — no cross-device comm needed).

Device algorithm per core (all matmuls in fp32r: full-rate, ~1e-4 rel):
  A) QKV projection with x^T resident in SBUF (fp32r), weights streamed.
     q/k produced transposed ([head_dim, L] layout) directly by using the
     weight block as the stationary operand; v in [L, head_dim] layout by
     using x^T as stationary.  Staged to DRAM.
  B) Per head pair: RoPE via a 64x64 signed-permutation matmul + elementwise
     combine; causal flash attention in S^T layout (keys on partitions),
     the two heads of a pair row-packed on the PE via tile_position:
     S^T stripes -> mask (diag only) -> exp (ACT) -> P^T V accumulation.
     Softmax denominator comes free from a ones-column appended to V (M=65);
     normalization applied to the context before the output projection
     (broadcast via a selector matmul + reciprocal + multiply).
  C) Output projection out_partial = ctx_norm @ wo_block, heads pair-stacked
     so the contraction is a full K=128.
"""

import os
import sys
import types

import numpy as np

# ---------------------------------------------------------------- constants
B, L, D = 2, 2048, 2048
HD = 64                 # head dim
N_HEADS = 32
LRPE_BASE = 512
N_CORES = 8
GROUPS = 4              # head groups (= cores per batch)
HPG = N_HEADS // GROUPS  # heads per core = 8
NEG = np.float32(-1e30)

_PROGRAM_CACHE = {}


def _env_setup():
    """Inject the NTFF-profile hook module the image's antenv lacks and stub
    the S3 artifact upload (no network in this container). Harmless if the
    real modules exist."""
    if "antenv.axon_hooks" not in sys.modules:
        mod = types.ModuleType("antenv.axon_hooks")
        mod._hook = None
        mod.set_axon_ntff_profile_hook = lambda h: setattr(mod, "_hook", h)
        mod.get_axon_ntff_profile_hook = lambda: mod._hook
        sys.modules["antenv.axon_hooks"] = mod
        try:
            import antenv
            antenv.axon_hooks = mod
        except ImportError:
            pass
    from antenv.axon_hooks import get_axon_ntff_profile_hook, set_axon_ntff_profile_hook
    if get_axon_ntff_profile_hook() is None:
        try:
            from trn_agent_boot.trn_boot import _ntff_profile_via_ctypes
            set_axon_ntff_profile_hook(
                _ntff_profile_via_ctypes("/opt/axon/libaxon_pjrt.so"))
        except Exception:
            pass
    import concourse.bass_utils as bu
    bu.upload_artifacts = lambda tmpdir: f"file://{tmpdir}"


# ---------------------------------------------------------------- program
def build_program(L_=L, D_=D, hpg=HPG):
    """Build the per-core Bass program (same SPMD program for all 8 cores)."""
    import concourse.bass as bass  # noqa: F401
    import concourse.mybir as mybir
    import concourse.tile as tile
    from concourse import bacc

    f32 = mybir.dt.float32
    f32r = mybir.dt.float32r
    DT = D_ // 128          # contraction chunks for the projections
    LT = L_ // 128          # l tiles
    ICS = L_ // 512         # query chunks of 512
    PAIRS = hpg // 2
    QK = hpg * HD           # q (or k) rows per core

    nc = bacc.Bacc()
    xT = nc.dram_tensor("xT", [D_, L_], f32, kind="ExternalInput")
    wqkT = nc.dram_tensor("wqkT", [D_, 2 * QK], f32, kind="ExternalInput")
    wvT = nc.dram_tensor("wvT", [D_, QK], f32, kind="ExternalInput")
    woT = nc.dram_tensor("woT", [128, PAIRS, D_], f32, kind="ExternalInput")
    cos2 = nc.dram_tensor("cos2", [128, L_], f32, kind="ExternalInput")
    sin2 = nc.dram_tensor("sin2", [128, L_], f32, kind="ExternalInput")
    perm = nc.dram_tensor("perm", [128, 128], f32, kind="ExternalInput")
    mask = nc.dram_tensor("mask", [128, 896], f32, kind="ExternalInput")
    out = nc.dram_tensor("out", [L_, D_], f32, kind="ExternalOutput")

    with tile.TileContext(nc) as tc:
        with tc.tile_pool(name="dram", bufs=1, space="DRAM") as dpool:
            qk_s = dpool.tile([2 * QK, L_], f32)      # staged q^T / k^T
            # staged v with a ones column per head: [L, hpg, 65]
            v_s = dpool.tile([L_, hpg, HD + 1], f32)

            # ================= Phase A: projections =================
            with (
                tc.tile_pool(name="xr", bufs=1) as xrpool,
                tc.tile_pool(name="stageA", bufs=2) as stpool,
                tc.tile_pool(name="outA", bufs=4) as oapool,
                tc.tile_pool(name="psumA", bufs=6, space="PSUM") as psA,
            ):
                xr = xrpool.tile([128, DT, L_], f32r)
                for dt in range(DT):
                    xs = stpool.tile([128, L_], f32, name="xstage")
                    nc.sync.dma_start(out=xs[:], in_=xT[128 * dt:128 * dt + 128, :])
                    nc.vector.tensor_copy(xr[:, dt, :], xs[:])

                # q/k projection: out tile [j_tile(128), l_chunk(512)]
                with tc.tile_pool(name="wqk", bufs=2) as wpool:
                    for jt in range(2 * QK // 128):
                        wf = wpool.tile([128, DT, 128], f32, name="w_f")
                        nc.sync.dma_start(
                            out=wf[:],
                            in_=wqkT[:, 128 * jt:128 * jt + 128].rearrange(
                                "(dt p) j -> p dt j", p=128))
                        wr = wpool.tile([128, DT, 128], f32r, name="w_r")
                        nc.vector.tensor_copy(wr[:], wf[:])
                        for lc in range(L_ // 512):
                            ps = psA.tile([128, 512], f32, name="psA")
                            for dt in range(DT):
                                nc.tensor.matmul(
                                    ps[:], wr[:, dt, :],
                                    xr[:, dt, 512 * lc:512 * lc + 512],
                                    start=(dt == 0), stop=(dt == DT - 1))
                            sb = oapool.tile([128, 512], f32, name="qk_sb")
                            nc.scalar.copy(sb[:], ps[:])
                            nc.sync.dma_start(
                                out=qk_s[128 * jt:128 * jt + 128,
                                         512 * lc:512 * lc + 512],
                                in_=sb[:])

                # v projection: out tile [l_tile(128), qk(<=512)], staged to
                # v_s interleaved with the per-head ones column
                with tc.tile_pool(name="wv", bufs=1) as wvpool:
                    wvr = wvpool.tile([128, DT, QK], f32r)
                    for dt in range(DT):
                        wvf = stpool.tile([128, QK], f32, name="wv_f")
                        nc.sync.dma_start(
                            out=wvf[:], in_=wvT[128 * dt:128 * dt + 128, :])
                        nc.vector.tensor_copy(wvr[:, dt, :], wvf[:])
                    for lt in range(LT):
                        ps = psA.tile([128, 512], f32, name="psA")
                        for dt in range(DT):
                            nc.tensor.matmul(
                                ps[:, :QK],
                                xr[:, dt, 128 * lt:128 * lt + 128],
                                wvr[:, dt, :],
                                start=(dt == 0), stop=(dt == DT - 1))
                        sb = oapool.tile([128, hpg, HD + 1], f32, name="v_sb")
                        nc.scalar.copy(
                            sb[:, :, 0:HD],
                            ps[:, :QK].rearrange("p (h d) -> p h d", d=HD))
                        nc.gpsimd.memset(sb[:, :, HD], 1.0)
                        nc.sync.dma_start(
                            out=v_s[128 * lt:128 * lt + 128, :, :], in_=sb[:])

            # ============ Phases B + C share the ctx_pair pool ============
            with (
                tc.tile_pool(name="consts", bufs=1) as cpool,
                tc.tile_pool(name="ctx_pair", bufs=1) as ctxpool,
            ):
                cos_sb = cpool.tile([128, L_], f32)
                sin_sb = cpool.tile([128, L_], f32)
                mask_sb = cpool.tile([128, 896], f32)
                nc.sync.dma_start(out=cos_sb[:], in_=cos2[:])
                nc.sync.dma_start(out=sin_sb[:], in_=sin2[:])
                nc.sync.dma_start(out=mask_sb[:], in_=mask[:])
                perm_f = cpool.tile([128, 128], f32)
                nc.sync.dma_start(out=perm_f[:], in_=perm[:])
                perm_r = cpool.tile([128, 128], f32r)
                nc.vector.tensor_copy(perm_r[:], perm_f[:])

                # head pair m: even head at partitions 0:64, odd at 64:128
                ctx_pair = ctxpool.tile([128, PAIRS, L_], f32r)

                # ================= Phase B: attention =================
                with (
                    tc.tile_pool(name="qkio", bufs=2) as iopool,
                    tc.tile_pool(name="qkr0", bufs=1) as r0pool,
                    tc.tile_pool(name="qkr", bufs=2) as qkrpool,
                    tc.tile_pool(name="ropetmp", bufs=2) as rtpool,
                    tc.tile_pool(name="pt", bufs=3) as ptpool,
                    tc.tile_pool(name="ctxsb", bufs=4) as cspool,
                    tc.tile_pool(name="norm", bufs=3) as nmpool,
                    tc.tile_pool(name="va", bufs=1) as vapool,
                    tc.tile_pool(name="psum_st", bufs=2, space="PSUM") as psST,
                    tc.tile_pool(name="psum_ctx", bufs=4, space="PSUM") as psCTX,
                ):
                    for t in range(PAIRS):
                        qp_f = iopool.tile([128, L_], f32, name="qp_f")
                        kp_f = iopool.tile([128, L_], f32, name="kp_f")
                        nc.sync.dma_start(out=qp_f[:],
                                          in_=qk_s[128 * t:128 * t + 128, :])
                        nc.sync.dma_start(
                            out=kp_f[:],
                            in_=qk_s[QK + 128 * t:QK + 128 * t + 128, :])
                        qp_r0 = r0pool.tile([128, L_], f32r, name="qp_r0")
                        kp_r0 = r0pool.tile([128, L_], f32r, name="kp_r0")
                        nc.vector.tensor_copy(qp_r0[:], qp_f[:])
                        nc.vector.tensor_copy(kp_r0[:], kp_f[:])
                        qp_r = qkrpool.tile([128, L_], f32r, name="qp_r")
                        kp_r = qkrpool.tile([128, L_], f32r, name="kp_r")
                        for (src_f, src_r0, dst) in ((qp_f, qp_r0, qp_r),
                                                     (kp_f, kp_r0, kp_r)):
                            for lc in range(L_ // 512):
                                s = slice(512 * lc, 512 * lc + 512)
                                rot = psST.tile([128, 1024], f32,
                                                name="st2")[:, :512]
                                nc.tensor.matmul(rot, perm_r[:], src_r0[:, s],
                                                 start=True, stop=True)
                                u1 = rtpool.tile([128, 512], f32, name="u1")
                                nc.vector.tensor_tensor(u1[:], rot, sin_sb[:, s],
                                                        mybir.AluOpType.mult)
                                u2 = rtpool.tile([128, 512], f32, name="u2")
                                nc.gpsimd.tensor_tensor(u2[:], src_f[:, s],
                                                        cos_sb[:, s],
                                                        mybir.AluOpType.mult)
                                nc.vector.tensor_tensor(dst[:, s], u1[:], u2[:],
                                                        mybir.AluOpType.add)

                        # v (with ones cols) for both heads of the pair
                        va = vapool.tile([128, LT, 2 * (HD + 1)], f32r,
                                         name="va")
                        vst = vapool.tile([128, LT, 2 * (HD + 1)], f32,
                                          name="vst")
                        nc.sync.dma_start(
                            out=vst[:],
                            in_=v_s[:, 2 * t:2 * t + 2, :].rearrange(
                                "(jt p) h d -> p jt (h d)", p=128))
                        nc.vector.tensor_copy(va[:], vst[:])

                        for ic in range(ICS):
                            i0 = 512 * ic
                            njt = 4 * ic + 4
                            ctx_ps = [psCTX.tile([65, 512], f32, name="ctx_ps")
                                      for _ in range(2)]
                            for jg in range(njt // 2):
                                st2 = [psST.tile([128, 1024], f32, name="st2")
                                       for _ in range(2)]
                                # interleave u-inner so adjacent matmuls hit
                                # disjoint PE row groups and run concurrently
                                for u in range(2):
                                    jt = 2 * jg + u
                                    for hh in range(2):
                                        hs = slice(64 * hh, 64 * hh + 64)
                                        nc.tensor.matmul(
                                            st2[hh][:, 512 * u:512 * u + 512],
                                            kp_r[hs, 128 * jt:128 * jt + 128],
                                            qp_r[hs, i0:i0 + 512],
                                            start=True, stop=True,
                                            tile_position=(64 * hh, 0))
                                for u in range(2):
                                    jt = 2 * jg + u
                                    o = 128 * jt - i0
                                    if o >= 0:  # diagonal stripe -> mask
                                        w = min(512, o + 128)
                                        for hh in range(2):
                                            nc.vector.tensor_tensor(
                                                st2[hh][:, 512 * u:512 * u + w],
                                                st2[hh][:, 512 * u:512 * u + w],
                                                mask_sb[:, 384 - o:384 - o + w],
                                                mybir.AluOpType.add)
                                pt2 = [ptpool.tile([128, 1024], f32r, name="pt2")
                                       for _ in range(2)]
                                for hh in range(2):
                                    nc.scalar.activation(
                                        pt2[hh][:], st2[hh][:],
                                        mybir.ActivationFunctionType.Exp)
                                for u in range(2):
                                    jt = 2 * jg + u
                                    for hh in range(2):
                                        nc.tensor.matmul(
                                            ctx_ps[hh][:],
                                            va[:, jt, 65 * hh:65 * hh + 65],
                                            pt2[hh][:, 512 * u:512 * u + 512],
                                            start=(jt == 0), stop=(jt == njt - 1))
                            for hh in range(2):
                                ctx_sb = cspool.tile([65, 512], f32r,
                                                     name="ctx_sb")
                                nc.scalar.copy(ctx_sb[:], ctx_ps[hh][:])
                                # 1/rowsum broadcast with no PE involvement:
                                # DMA the rowsum row to partition 0, DVE
                                # reciprocal, GPSIMD partition-broadcast.
                                rsrow = nmpool.tile([1, 512], f32, name="rsrow")
                                nc.gpsimd.dma_start(out=rsrow[:],
                                                    in_=ctx_sb[64:65, :].bitcast(f32))
                                nc.vector.reciprocal(rsrow[:], rsrow[:])
                                rec = nmpool.tile([64, 512], f32, name="rec")
                                nc.gpsimd.partition_broadcast(rec[:], rsrow[:])
                                if hh == 0:
                                    nc.gpsimd.tensor_tensor(
                                        ctx_pair[0:64, t, i0:i0 + 512],
                                        ctx_sb[0:64, :], rec[:],
                                        mybir.AluOpType.mult)
                                else:
                                    # odd head lives at partitions 64:128 of
                                    # ctx_pair; engines cannot shift partitions,
                                    # so: f32 multiply -> sbuf-to-sbuf DMA shift
                                    # -> fp32r rounding copy at 64:128.
                                    ctmp = nmpool.tile([64, 512], f32,
                                                       name="ctmp")
                                    nc.gpsimd.tensor_tensor(
                                        ctmp[:], ctx_sb[0:64, :], rec[:],
                                        mybir.AluOpType.mult)
                                    cst = nmpool.tile([128, 512], f32,
                                                      name="cstage")
                                    nc.gpsimd.dma_start(out=cst[64:128, :],
                                                        in_=ctmp[:])
                                    nc.vector.tensor_copy(
                                        ctx_pair[64:128, t, i0:i0 + 512],
                                        cst[64:128, :])

                # ================= Phase C: output projection =================
                with (
                    tc.tile_pool(name="wo", bufs=1) as wopool,
                    tc.tile_pool(name="outC", bufs=4) as ocpool,
                    tc.tile_pool(name="psumC", bufs=6, space="PSUM") as psC,
                ):
                    wof = wopool.tile([128, PAIRS, D_], f32)
                    nc.sync.dma_start(out=wof[:], in_=woT[:])
                    wor = wopool.tile([128, PAIRS, D_], f32r)
                    nc.vector.tensor_copy(wor[:], wof[:])
                    for lt in range(LT):
                        for oc in range(D_ // 512):
                            ps = psC.tile([128, 512], f32, name="psC")
                            for m in range(PAIRS):
                                nc.tensor.matmul(
                                    ps[:], ctx_pair[:, m, 128 * lt:128 * lt + 128],
                                    wor[:, m, 512 * oc:512 * oc + 512],
                                    start=(m == 0), stop=(m == PAIRS - 1))
                            osb = ocpool.tile([128, 512], f32, name="osb")
                            nc.scalar.copy(osb[:], ps[:])
                            nc.sync.dma_start(
                                out=out[128 * lt:128 * lt + 128,
                                        512 * oc:512 * oc + 512],
                                in_=osb[:])

    nc.finalize()
    return nc


# ---------------------------------------------------------------- host prep
def _host_constants(L_=L):
    inv_freq = 1.0 / (LRPE_BASE ** (np.arange(0, HD, 2, dtype=np.float32) / HD))
    t = np.arange(L_, dtype=np.float32)
    freqs = t[:, None] * inv_freq[None, :]
    emb = np.concatenate([freqs, freqs], axis=-1)          # (L, 64)
    cosT = np.cos(emb).T.astype(np.float32)                # (64, L)
    sinT = np.sin(emb).T.astype(np.float32)
    cos2 = np.ascontiguousarray(np.tile(cosT, (2, 1)))     # (128, L)
    sin2 = np.ascontiguousarray(np.tile(sinT, (2, 1)))
    perm64 = np.zeros((64, 64), dtype=np.float32)
    for dp in range(32):
        perm64[dp + 32, dp] = -1.0
    for dp in range(32, 64):
        perm64[dp - 32, dp] = 1.0
    perm = np.zeros((128, 128), dtype=np.float32)
    perm[0:64, 0:64] = perm64
    perm[64:128, 64:128] = perm64
    maskbig = np.where(
        np.arange(896)[None, :] < np.arange(128)[:, None] + 384, NEG,
        np.float32(0.0)).astype(np.float32)
    return cos2, sin2, perm, maskbig


def _in_maps(x, wqkv, wo):
    cos2, sin2, perm, maskbig = _host_constants()
    scale = np.float32(HD ** -0.5)
    xT = [np.ascontiguousarray(x[b].T) for b in range(B)]
    maps = []
    for c in range(N_CORES):
        b, g = c // GROUPS, c % GROUPS
        r0 = HD * HPG * g
        wq = wqkv[r0:r0 + HD * HPG] * scale
        wk = wqkv[D + r0:D + r0 + HD * HPG]
        wv = wqkv[2 * D + r0:2 * D + r0 + HD * HPG]
        # wo block columns for this group, pair-stacked: (128, PAIRS, D)
        wo_blk = wo[:, HD * HPG * g:HD * HPG * (g + 1)].T  # (512, D), row=64h+d
        wo_pair = np.ascontiguousarray(
            wo_blk.reshape(HPG // 2, 128, D).transpose(1, 0, 2))
        maps.append({
            "xT": xT[b],
            "wqkT": np.ascontiguousarray(np.concatenate([wq, wk], 0).T),
            "wvT": np.ascontiguousarray(wv.T),
            "woT": wo_pair,
            "cos2": cos2, "sin2": sin2, "perm": perm, "mask": maskbig,
        })
    return maps


def run(x, wqkv, wo, trace=False):
    _env_setup()
    from concourse.bass_utils import run_bass_kernel_spmd

    if "prog" not in _PROGRAM_CACHE:
        _PROGRAM_CACHE["prog"] = build_program()
    nc = _PROGRAM_CACHE["prog"]
    maps = _in_maps(np.asarray(x), np.asarray(wqkv), np.asarray(wo))
    res = run_bass_kernel_spmd(nc, maps, list(range(N_CORES)), trace=trace)
    out = np.zeros((B, L, D), dtype=np.float32)
    for c in range(N_CORES):
        out[c // GROUPS] += res.results[c]["out"]
    return out, res


def kernel(x, wqkv, wo):
    out, _ = run(x, wqkv, wo, trace=False)
    return out


# revision 10
# speedup vs baseline: 1.3832x; 1.3832x over previous
"""Trainium2 Bass kernel for nn_Attention_58798102283008.

Dense causal transformer attention block:
    qkv = x @ wqkv.T ; RoPE(q, k) ; causal softmax attention ; out @ wo.T
Shapes: x (2, 2048, 2048), wqkv (6144, 2048), wo (2048, 2048), f32.

Sharding over 8 NeuronCores: data parallel on batch (2) x tensor parallel on
heads (32 heads -> 4 groups of 8). Core c handles batch c//4, head group c%4.
Each core computes its heads' attention and a partial output projection
(wo column block); the host sums the 4 partials per batch (the unshard step
for this tensor-parallel decomposition — no cross-device comm needed).

Device algorithm per core (all matmuls in fp32r: full-rate, ~1e-4 rel):
  A) QKV projection with x^T resident in SBUF (fp32r), weights streamed.
     q/k produced transposed ([head_dim, L] layout) directly by using the
     weight block as the stationary operand; v in [L, head_dim] layout by
     using x^T as stationary.  Staged to DRAM.
  B) Per head pair: RoPE via a 64x64 signed-permutation matmul + elementwise
     combine; causal flash attention in S^T layout (keys on partitions),
     the two heads of a pair row-packed on the PE via tile_position:
     S^T stripes -> mask (diag only) -> exp (ACT) -> P^T V accumulation.
     Softmax denominator comes free from a ones-column appended to V (M=65);
     normalization applied to the context before the output projection
     (broadcast via a selector matmul + reciprocal + multiply).
  C) Output projection out_partial = ctx_norm @ wo_block, heads pair-stacked
     so the contraction is a full K=128.
"""

import os
import sys
import types

import numpy as np

# ---------------------------------------------------------------- constants
B, L, D = 2, 2048, 2048
HD = 64                 # head dim
N_HEADS = 32
LRPE_BASE = 512
N_CORES = 8
GROUPS = 4              # head groups (= cores per batch)
HPG = N_HEADS // GROUPS  # heads per core = 8
NEG = np.float32(-1e30)

_PROGRAM_CACHE = {}


def _env_setup():
    """Inject the NTFF-profile hook module the image's antenv lacks and stub
    the S3 artifact upload (no network in this container). Harmless if the
    real modules exist."""
    if "antenv.axon_hooks" not in sys.modules:
        mod = types.ModuleType("antenv.axon_hooks")
        mod._hook = None
        mod.set_axon_ntff_profile_hook = lambda h: setattr(mod, "_hook", h)
        mod.get_axon_ntff_profile_hook = lambda: mod._hook
        sys.modules["antenv.axon_hooks"] = mod
        try:
            import antenv
            antenv.axon_hooks = mod
        except ImportError:
            pass
    from antenv.axon_hooks import get_axon_ntff_profile_hook, set_axon_ntff_profile_hook
    if get_axon_ntff_profile_hook() is None:
        try:
            from trn_agent_boot.trn_boot import _ntff_profile_via_ctypes
            set_axon_ntff_profile_hook(
                _ntff_profile_via_ctypes("/opt/axon/libaxon_pjrt.so"))
        except Exception:
            pass
    import concourse.bass_utils as bu
    bu.upload_artifacts = lambda tmpdir: f"file://{tmpdir}"


# ---------------------------------------------------------------- program
def build_program(L_=L, D_=D, hpg=HPG):
    """Build the per-core Bass program (same SPMD program for all 8 cores)."""
    import concourse.bass as bass  # noqa: F401
    import concourse.mybir as mybir
    import concourse.tile as tile
    from concourse import bacc

    f32 = mybir.dt.float32
    f32r = mybir.dt.float32r
    DT = D_ // 128          # contraction chunks for the projections
    LT = L_ // 128          # l tiles
    ICS = L_ // 512         # query chunks of 512
    PAIRS = hpg // 2
    QK = hpg * HD           # q (or k) rows per core

    nc = bacc.Bacc()
    xT = nc.dram_tensor("xT", [D_, L_], f32, kind="ExternalInput")
    wqkT = nc.dram_tensor("wqkT", [D_, 2 * QK], f32, kind="ExternalInput")
    wvT = nc.dram_tensor("wvT", [D_, QK], f32, kind="ExternalInput")
    woT = nc.dram_tensor("woT", [128, PAIRS, D_], f32, kind="ExternalInput")
    cos2 = nc.dram_tensor("cos2", [128, L_], f32, kind="ExternalInput")
    sin2 = nc.dram_tensor("sin2", [128, L_], f32, kind="ExternalInput")
    perm = nc.dram_tensor("perm", [128, 128], f32, kind="ExternalInput")
    mask = nc.dram_tensor("mask", [128, 896], f32, kind="ExternalInput")
    out = nc.dram_tensor("out", [L_, D_], f32, kind="ExternalOutput")

    with tile.TileContext(nc) as tc:
        with tc.tile_pool(name="dram", bufs=1, space="DRAM") as dpool:
            qk_s = dpool.tile([2 * QK, L_], f32)      # staged q^T / k^T
            # staged v with a ones column per head: [L, hpg, 65]
            v_s = dpool.tile([L_, hpg, HD + 1], f32)

            # ================= Phase A: projections =================
            with (
                tc.tile_pool(name="xr", bufs=1) as xrpool,
                tc.tile_pool(name="stageA", bufs=2) as stpool,
                tc.tile_pool(name="outA", bufs=4) as oapool,
                tc.tile_pool(name="psumA", bufs=6, space="PSUM") as psA,
            ):
                xr = xrpool.tile([128, DT, L_], f32r)
                for dt in range(DT):
                    xs = stpool.tile([128, L_], f32, name="xstage")
                    nc.sync.dma_start(out=xs[:], in_=xT[128 * dt:128 * dt + 128, :])
                    nc.vector.tensor_copy(xr[:, dt, :], xs[:])

                # q/k projection: out tile [j_tile(128), l_chunk(512)]
                with tc.tile_pool(name="wqk", bufs=2) as wpool:
                    for jt in range(2 * QK // 128):
                        wf = wpool.tile([128, DT, 128], f32, name="w_f")
                        nc.sync.dma_start(
                            out=wf[:],
                            in_=wqkT[:, 128 * jt:128 * jt + 128].rearrange(
                                "(dt p) j -> p dt j", p=128))
                        wr = wpool.tile([128, DT, 128], f32r, name="w_r")
                        nc.vector.tensor_copy(wr[:], wf[:])
                        for lc in range(L_ // 512):
                            ps = psA.tile([128, 512], f32, name="psA")
                            for dt in range(DT):
                                nc.tensor.matmul(
                                    ps[:], wr[:, dt, :],
                                    xr[:, dt, 512 * lc:512 * lc + 512],
                                    start=(dt == 0), stop=(dt == DT - 1))
                            sb = oapool.tile([128, 512], f32, name="qk_sb")
                            nc.scalar.copy(sb[:], ps[:])
                            nc.sync.dma_start(
                                out=qk_s[128 * jt:128 * jt + 128,
                                         512 * lc:512 * lc + 512],
                                in_=sb[:])

                # v projection: out tile [l_tile(128), qk(<=512)], staged to
                # v_s interleaved with the per-head ones column
                with tc.tile_pool(name="wv", bufs=1) as wvpool:
                    wvr = wvpool.tile([128, DT, QK], f32r)
                    for dt in range(DT):
                        wvf = stpool.tile([128, QK], f32, name="wv_f")
                        nc.sync.dma_start(
                            out=wvf[:], in_=wvT[128 * dt:128 * dt + 128, :])
                        nc.vector.tensor_copy(wvr[:, dt, :], wvf[:])
                    for lt in range(LT):
                        ps = psA.tile([128, 512], f32, name="psA")
                        for dt in range(DT):
                            nc.tensor.matmul(
                                ps[:, :QK],
                                xr[:, dt, 128 * lt:128 * lt + 128],
                                wvr[:, dt, :],
                                start=(dt == 0), stop=(dt == DT - 1))
                        sb = oapool.tile([128, hpg, HD + 1], f32, name="v_sb")
                        nc.scalar.copy(
                            sb[:, :, 0:HD],
                            ps[:, :QK].rearrange("p (h d) -> p h d", d=HD))
                        nc.gpsimd.memset(sb[:, :, HD], 1.0)
                        nc.sync.dma_start(
                            out=v_s[128 * lt:128 * lt + 128, :, :], in_=sb[:])

            # ============ Phases B + C share the ctx_pair pool ============
            with (
                tc.tile_pool(name="consts", bufs=1) as cpool,
                tc.tile_pool(name="ctx_pair", bufs=1) as ctxpool,
            ):
                cos_sb = cpool.tile([128, L_], f32)
                sin_sb = cpool.tile([128, L_], f32)
                mask_sb = cpool.tile([128, 896], f32)
                nc.sync.dma_start(out=cos_sb[:], in_=cos2[:])
                nc.sync.dma_start(out=sin_sb[:], in_=sin2[:])
                nc.sync.dma_start(out=mask_sb[:], in_=mask[:])
                perm_f = cpool.tile([128, 128], f32)
                nc.sync.dma_start(out=perm_f[:], in_=perm[:])
                perm_r = cpool.tile([128, 128], f32r)
                nc.vector.tensor_copy(perm_r[:], perm_f[:])

                # head pair m: even head at partitions 0:64, odd at 64:128
                ctx_pair = ctxpool.tile([128, PAIRS, L_], f32r)

                # ================= Phase B: attention =================
                with (
                    tc.tile_pool(name="qkio", bufs=2) as iopool,
                    tc.tile_pool(name="qkr0", bufs=1) as r0pool,
                    tc.tile_pool(name="qkr", bufs=2) as qkrpool,
                    tc.tile_pool(name="ropetmp", bufs=2) as rtpool,
                    tc.tile_pool(name="pt", bufs=3) as ptpool,
                    tc.tile_pool(name="ctxsb", bufs=4) as cspool,
                    tc.tile_pool(name="norm", bufs=3) as nmpool,
                    tc.tile_pool(name="va", bufs=1) as vapool,
                    tc.tile_pool(name="psum_st", bufs=2, space="PSUM") as psST,
                    tc.tile_pool(name="psum_ctx", bufs=4, space="PSUM") as psCTX,
                ):
                    pend = []

                    def emit_norm(ctx_sb, hh, t, i0):
                        # 1/rowsum broadcast with no PE involvement: DMA the
                        # rowsum row to partition 0, DVE reciprocal, GPSIMD
                        # partition-broadcast, then normalize the context.
                        rsrow = nmpool.tile([1, 512], f32, name="rsrow")
                        nc.sync.dma_start(out=rsrow[:],
                                          in_=ctx_sb[64:65, :].bitcast(f32))
                        nc.vector.reciprocal(rsrow[:], rsrow[:])
                        rec = nmpool.tile([64, 512], f32, name="rec")
                        nc.gpsimd.partition_broadcast(rec[:], rsrow[:])
                        if hh == 0:
                            nc.vector.tensor_tensor(
                                ctx_pair[0:64, t, i0:i0 + 512],
                                ctx_sb[0:64, :], rec[:],
                                mybir.AluOpType.mult)
                        else:
                            # odd head lives at partitions 64:128 of ctx_pair;
                            # engines cannot shift partitions, so: f32 multiply
                            # -> sbuf-to-sbuf DMA shift -> fp32r rounding copy.
                            ctmp = nmpool.tile([64, 512], f32, name="ctmp")
                            nc.vector.tensor_tensor(
                                ctmp[:], ctx_sb[0:64, :], rec[:],
                                mybir.AluOpType.mult)
                            cst = nmpool.tile([128, 512], f32, name="cstage")
                            nc.sync.dma_start(out=cst[64:128, :], in_=ctmp[:])
                            nc.vector.tensor_copy(
                                ctx_pair[64:128, t, i0:i0 + 512],
                                cst[64:128, :])

                    for t in range(PAIRS):
                        qp_f = iopool.tile([128, L_], f32, name="qp_f")
                        kp_f = iopool.tile([128, L_], f32, name="kp_f")
                        nc.sync.dma_start(out=qp_f[:],
                                          in_=qk_s[128 * t:128 * t + 128, :])
                        nc.sync.dma_start(
                            out=kp_f[:],
                            in_=qk_s[QK + 128 * t:QK + 128 * t + 128, :])
                        qp_r0 = r0pool.tile([128, L_], f32r, name="qp_r0")
                        kp_r0 = r0pool.tile([128, L_], f32r, name="kp_r0")
                        nc.vector.tensor_copy(qp_r0[:], qp_f[:])
                        nc.vector.tensor_copy(kp_r0[:], kp_f[:])
                        qp_r = qkrpool.tile([128, L_], f32r, name="qp_r")
                        kp_r = qkrpool.tile([128, L_], f32r, name="kp_r")
                        for (src_f, src_r0, dst) in ((qp_f, qp_r0, qp_r),
                                                     (kp_f, kp_r0, kp_r)):
                            for lc in range(L_ // 512):
                                s = slice(512 * lc, 512 * lc + 512)
                                rot = psST.tile([128, 1024], f32,
                                                name="st2")[:, :512]
                                nc.tensor.matmul(rot, perm_r[:], src_r0[:, s],
                                                 start=True, stop=True)
                                u1 = rtpool.tile([128, 512], f32, name="u1")
                                nc.vector.tensor_tensor(u1[:], rot, sin_sb[:, s],
                                                        mybir.AluOpType.mult)
                                u2 = rtpool.tile([128, 512], f32, name="u2")
                                nc.gpsimd.tensor_tensor(u2[:], src_f[:, s],
                                                        cos_sb[:, s],
                                                        mybir.AluOpType.mult)
                                nc.vector.tensor_tensor(dst[:, s], u1[:], u2[:],
                                                        mybir.AluOpType.add)

                        # v (with ones cols) for both heads of the pair
                        va = vapool.tile([128, LT, 2 * (HD + 1)], f32r,
                                         name="va")
                        vst = vapool.tile([128, LT, 2 * (HD + 1)], f32,
                                          name="vst")
                        nc.sync.dma_start(
                            out=vst[:],
                            in_=v_s[:, 2 * t:2 * t + 2, :].rearrange(
                                "(jt p) h d -> p jt (h d)", p=128))
                        nc.vector.tensor_copy(va[:], vst[:])

                        for ic in range(ICS):
                            i0 = 512 * ic
                            njt = 4 * ic + 4
                            ctx_ps = [psCTX.tile([65, 512], f32, name="ctx_ps")
                                      for _ in range(2)]
                            pend_new = []
                            for jg in range(njt // 2):
                                st2 = [psST.tile([128, 1024], f32, name="st2")
                                       for _ in range(2)]
                                # interleave u-inner so adjacent matmuls hit
                                # disjoint PE row groups and run concurrently
                                for u in range(2):
                                    jt = 2 * jg + u
                                    for hh in range(2):
                                        hs = slice(64 * hh, 64 * hh + 64)
                                        nc.tensor.matmul(
                                            st2[hh][:, 512 * u:512 * u + 512],
                                            kp_r[hs, 128 * jt:128 * jt + 128],
                                            qp_r[hs, i0:i0 + 512],
                                            start=True, stop=True,
                                            tile_position=(64 * hh, 0))
                                for u in range(2):
                                    jt = 2 * jg + u
                                    o = 128 * jt - i0
                                    if o >= 0:  # diagonal stripe -> mask
                                        w = min(512, o + 128)
                                        for hh in range(2):
                                            nc.vector.tensor_tensor(
                                                st2[hh][:, 512 * u:512 * u + w],
                                                st2[hh][:, 512 * u:512 * u + w],
                                                mask_sb[:, 384 - o:384 - o + w],
                                                mybir.AluOpType.add)
                                pt2 = [ptpool.tile([128, 1024], f32r, name="pt2")
                                       for _ in range(2)]
                                for hh in range(2):
                                    nc.scalar.activation(
                                        pt2[hh][:], st2[hh][:],
                                        mybir.ActivationFunctionType.Exp)
                                for u in range(2):
                                    jt = 2 * jg + u
                                    for hh in range(2):
                                        nc.tensor.matmul(
                                            ctx_ps[hh][:],
                                            va[:, jt, 65 * hh:65 * hh + 65],
                                            pt2[hh][:, 512 * u:512 * u + 512],
                                            start=(jt == 0), stop=(jt == njt - 1))
                            for hh in range(2):
                                ctx_sb = cspool.tile([65, 512], f32r,
                                                     name="ctx_sb")
                                nc.scalar.copy(ctx_sb[:], ctx_ps[hh][:])
                                pend_new.append((ctx_sb, hh, t, i0))
                            # emit the previous chunk's normalization now —
                            # one step behind, so these latency chains fill
                            # engine idle slots instead of blocking the next
                            # chunk's matmul-feeding work (in-order queues).
                            for p in pend:
                                emit_norm(*p)
                            pend = pend_new

                    for p in pend:
                        emit_norm(*p)

                # ================= Phase C: output projection =================
                with (
                    tc.tile_pool(name="wo", bufs=1) as wopool,
                    tc.tile_pool(name="outC", bufs=4) as ocpool,
                    tc.tile_pool(name="psumC", bufs=6, space="PSUM") as psC,
                ):
                    wof = wopool.tile([128, PAIRS, D_], f32)
                    nc.sync.dma_start(out=wof[:], in_=woT[:])
                    wor = wopool.tile([128, PAIRS, D_], f32r)
                    nc.vector.tensor_copy(wor[:], wof[:])
                    for lt in range(LT):
                        for oc in range(D_ // 512):
                            ps = psC.tile([128, 512], f32, name="psC")
                            for m in range(PAIRS):
                                nc.tensor.matmul(
                                    ps[:], ctx_pair[:, m, 128 * lt:128 * lt + 128],
                                    wor[:, m, 512 * oc:512 * oc + 512],
                                    start=(m == 0), stop=(m == PAIRS - 1))
                            osb = ocpool.tile([128, 512], f32, name="osb")
                            nc.scalar.copy(osb[:], ps[:])
                            nc.sync.dma_start(
                                out=out[128 * lt:128 * lt + 128,
                                        512 * oc:512 * oc + 512],
                                in_=osb[:])

    nc.finalize()
    return nc


# ---------------------------------------------------------------- host prep
def _host_constants(L_=L):
    inv_freq = 1.0 / (LRPE_BASE ** (np.arange(0, HD, 2, dtype=np.float32) / HD))
    t = np.arange(L_, dtype=np.float32)
    freqs = t[:, None] * inv_freq[None, :]
    emb = np.concatenate([freqs, freqs], axis=-1)          # (L, 64)
    cosT = np.cos(emb).T.astype(np.float32)                # (64, L)
    sinT = np.sin(emb).T.astype(np.float32)
    cos2 = np.ascontiguousarray(np.tile(cosT, (2, 1)))     # (128, L)
    sin2 = np.ascontiguousarray(np.tile(sinT, (2, 1)))
    perm64 = np.zeros((64, 64), dtype=np.float32)
    for dp in range(32):
        perm64[dp + 32, dp] = -1.0
    for dp in range(32, 64):
        perm64[dp - 32, dp] = 1.0
    perm = np.zeros((128, 128), dtype=np.float32)
    perm[0:64, 0:64] = perm64
    perm[64:128, 64:128] = perm64
    maskbig = np.where(
        np.arange(896)[None, :] < np.arange(128)[:, None] + 384, NEG,
        np.float32(0.0)).astype(np.float32)
    return cos2, sin2, perm, maskbig


def _in_maps(x, wqkv, wo):
    cos2, sin2, perm, maskbig = _host_constants()
    scale = np.float32(HD ** -0.5)
    xT = [np.ascontiguousarray(x[b].T) for b in range(B)]
    maps = []
    for c in range(N_CORES):
        b, g = c // GROUPS, c % GROUPS
        r0 = HD * HPG * g
        wq = wqkv[r0:r0 + HD * HPG] * scale
        wk = wqkv[D + r0:D + r0 + HD * HPG]
        wv = wqkv[2 * D + r0:2 * D + r0 + HD * HPG]
        # wo block columns for this group, pair-stacked: (128, PAIRS, D)
        wo_blk = wo[:, HD * HPG * g:HD * HPG * (g + 1)].T  # (512, D), row=64h+d
        wo_pair = np.ascontiguousarray(
            wo_blk.reshape(HPG // 2, 128, D).transpose(1, 0, 2))
        maps.append({
            "xT": xT[b],
            "wqkT": np.ascontiguousarray(np.concatenate([wq, wk], 0).T),
            "wvT": np.ascontiguousarray(wv.T),
            "woT": wo_pair,
            "cos2": cos2, "sin2": sin2, "perm": perm, "mask": maskbig,
        })
    return maps


def run(x, wqkv, wo, trace=False):
    _env_setup()
    from concourse.bass_utils import run_bass_kernel_spmd

    if "prog" not in _PROGRAM_CACHE:
        _PROGRAM_CACHE["prog"] = build_program()
    nc = _PROGRAM_CACHE["prog"]
    maps = _in_maps(np.asarray(x), np.asarray(wqkv), np.asarray(wo))
    res = run_bass_kernel_spmd(nc, maps, list(range(N_CORES)), trace=trace)
    out = np.zeros((B, L, D), dtype=np.float32)
    for c in range(N_CORES):
        out[c // GROUPS] += res.results[c]["out"]
    return out, res


def kernel(x, wqkv, wo):
    out, _ = run(x, wqkv, wo, trace=False)
    return out


# revision 12
# speedup vs baseline: 1.4652x; 1.0593x over previous
"""Trainium2 Bass kernel for nn_Attention_58798102283008.

Dense causal transformer attention block:
    qkv = x @ wqkv.T ; RoPE(q, k) ; causal softmax attention ; out @ wo.T
Shapes: x (2, 2048, 2048), wqkv (6144, 2048), wo (2048, 2048), f32.

Sharding over 8 NeuronCores: data parallel on batch (2) x tensor parallel on
heads (32 heads -> 4 groups of 8). Core c handles batch c//4, head group c%4.
Each core computes its heads' attention and a partial output projection
(wo column block); the host sums the 4 partials per batch (the unshard step
for this tensor-parallel decomposition — no cross-device comm needed).

Device algorithm per core (all matmuls in fp32r: full-rate, ~1e-4 rel):
  A) QKV projection with x^T resident in SBUF (fp32r), weights streamed.
     q/k produced transposed ([head_dim, L] layout) directly by using the
     weight block as the stationary operand; v in [L, head_dim] layout by
     using x^T as stationary.  Staged to DRAM.
  B) Per head pair: RoPE via a 64x64 signed-permutation matmul + elementwise
     combine; causal flash attention in S^T layout (keys on partitions),
     the two heads of a pair row-packed on the PE via tile_position:
     S^T stripes -> mask (diag only) -> exp (ACT) -> P^T V accumulation.
     Softmax denominator comes free from a ones-column appended to V (M=65);
     normalization applied to the context before the output projection
     (broadcast via a selector matmul + reciprocal + multiply).
  C) Output projection out_partial = ctx_norm @ wo_block, heads pair-stacked
     so the contraction is a full K=128.
"""

import os
import sys
import types

import numpy as np

# ---------------------------------------------------------------- constants
B, L, D = 2, 2048, 2048
HD = 64                 # head dim
N_HEADS = 32
LRPE_BASE = 512
N_CORES = 8
GROUPS = 4              # head groups (= cores per batch)
HPG = N_HEADS // GROUPS  # heads per core = 8
NEG = np.float32(-1e30)

_PROGRAM_CACHE = {}


def _env_setup():
    """Inject the NTFF-profile hook module the image's antenv lacks and stub
    the S3 artifact upload (no network in this container). Harmless if the
    real modules exist."""
    if "antenv.axon_hooks" not in sys.modules:
        mod = types.ModuleType("antenv.axon_hooks")
        mod._hook = None
        mod.set_axon_ntff_profile_hook = lambda h: setattr(mod, "_hook", h)
        mod.get_axon_ntff_profile_hook = lambda: mod._hook
        sys.modules["antenv.axon_hooks"] = mod
        try:
            import antenv
            antenv.axon_hooks = mod
        except ImportError:
            pass
    from antenv.axon_hooks import get_axon_ntff_profile_hook, set_axon_ntff_profile_hook
    if get_axon_ntff_profile_hook() is None:
        try:
            from trn_agent_boot.trn_boot import _ntff_profile_via_ctypes
            set_axon_ntff_profile_hook(
                _ntff_profile_via_ctypes("/opt/axon/libaxon_pjrt.so"))
        except Exception:
            pass
    import concourse.bass_utils as bu
    bu.upload_artifacts = lambda tmpdir: f"file://{tmpdir}"


# ---------------------------------------------------------------- program
def build_program(L_=L, D_=D, hpg=HPG):
    """Build the per-core Bass program (same SPMD program for all 8 cores)."""
    import concourse.bass as bass  # noqa: F401
    import concourse.mybir as mybir
    import concourse.tile as tile
    from concourse import bacc

    f32 = mybir.dt.float32
    f32r = mybir.dt.float32r
    DT = D_ // 128          # contraction chunks for the projections
    LT = L_ // 128          # l tiles
    ICS = L_ // 512         # query chunks of 512
    PAIRS = hpg // 2
    QK = hpg * HD           # q (or k) rows per core

    nc = bacc.Bacc()
    xT = nc.dram_tensor("xT", [D_, L_], f32, kind="ExternalInput")
    wqkT = nc.dram_tensor("wqkT", [D_, 2 * QK], f32, kind="ExternalInput")
    wvT = nc.dram_tensor("wvT", [D_, QK], f32, kind="ExternalInput")
    woT = nc.dram_tensor("woT", [128, PAIRS, D_], f32, kind="ExternalInput")
    cos2 = nc.dram_tensor("cos2", [128, L_], f32, kind="ExternalInput")
    sin2 = nc.dram_tensor("sin2", [128, L_], f32, kind="ExternalInput")
    perm = nc.dram_tensor("perm", [128, 128], f32, kind="ExternalInput")
    mask = nc.dram_tensor("mask", [128, 896], f32, kind="ExternalInput")
    out = nc.dram_tensor("out", [L_, D_], f32, kind="ExternalOutput")

    with tile.TileContext(nc) as tc:
        with tc.tile_pool(name="dram", bufs=1, space="DRAM") as dpool:
            qk_s = dpool.tile([2 * QK, L_], f32)      # staged q^T / k^T
            # staged v with a ones column per head: [L, hpg, 65]
            v_s = dpool.tile([L_, hpg, HD + 1], f32)

            # ================= Phase A: projections =================
            with (
                tc.tile_pool(name="xr", bufs=1) as xrpool,
                tc.tile_pool(name="stageA", bufs=2) as stpool,
                tc.tile_pool(name="outA", bufs=4) as oapool,
                tc.tile_pool(name="psumA", bufs=6, space="PSUM") as psA,
            ):
                xr = xrpool.tile([128, DT, L_], f32r)
                for dt in range(DT):
                    xs = stpool.tile([128, L_], f32, name="xstage")
                    nc.sync.dma_start(out=xs[:], in_=xT[128 * dt:128 * dt + 128, :])
                    nc.vector.tensor_copy(xr[:, dt, :], xs[:])

                # q/k projection: out tile [j_tile(128), l_chunk(512)]
                with tc.tile_pool(name="wqk", bufs=2) as wpool:
                    for jt in range(2 * QK // 128):
                        wf = wpool.tile([128, DT, 128], f32, name="w_f")
                        nc.sync.dma_start(
                            out=wf[:],
                            in_=wqkT[:, 128 * jt:128 * jt + 128].rearrange(
                                "(dt p) j -> p dt j", p=128))
                        wr = wpool.tile([128, DT, 128], f32r, name="w_r")
                        nc.vector.tensor_copy(wr[:], wf[:])
                        for lc in range(L_ // 512):
                            ps = psA.tile([128, 512], f32, name="psA")
                            for dt in range(DT):
                                nc.tensor.matmul(
                                    ps[:], wr[:, dt, :],
                                    xr[:, dt, 512 * lc:512 * lc + 512],
                                    start=(dt == 0), stop=(dt == DT - 1))
                            sb = oapool.tile([128, 512], f32, name="qk_sb")
                            nc.scalar.copy(sb[:], ps[:])
                            nc.sync.dma_start(
                                out=qk_s[128 * jt:128 * jt + 128,
                                         512 * lc:512 * lc + 512],
                                in_=sb[:])

                # v projection: out tile [l_tile(128), qk(<=512)], staged to
                # v_s interleaved with the per-head ones column
                with tc.tile_pool(name="wv", bufs=1) as wvpool:
                    wvr = wvpool.tile([128, DT, QK], f32r)
                    for dt in range(DT):
                        wvf = stpool.tile([128, QK], f32, name="wv_f")
                        nc.sync.dma_start(
                            out=wvf[:], in_=wvT[128 * dt:128 * dt + 128, :])
                        nc.vector.tensor_copy(wvr[:, dt, :], wvf[:])
                    for lt in range(LT):
                        ps = psA.tile([128, 512], f32, name="psA")
                        for dt in range(DT):
                            nc.tensor.matmul(
                                ps[:, :QK],
                                xr[:, dt, 128 * lt:128 * lt + 128],
                                wvr[:, dt, :],
                                start=(dt == 0), stop=(dt == DT - 1))
                        sb = oapool.tile([128, hpg, HD + 1], f32, name="v_sb")
                        nc.scalar.copy(
                            sb[:, :, 0:HD],
                            ps[:, :QK].rearrange("p (h d) -> p h d", d=HD))
                        nc.gpsimd.memset(sb[:, :, HD], 1.0)
                        nc.sync.dma_start(
                            out=v_s[128 * lt:128 * lt + 128, :, :], in_=sb[:])

            # ============ Phases B + C share the ctx_pair pool ============
            with (
                tc.tile_pool(name="consts", bufs=1) as cpool,
                tc.tile_pool(name="ctx_pair", bufs=1) as ctxpool,
            ):
                cos_sb = cpool.tile([128, L_], f32)
                sin_sb = cpool.tile([128, L_], f32)
                mask_sb = cpool.tile([128, 896], f32)
                nc.sync.dma_start(out=cos_sb[:], in_=cos2[:])
                nc.sync.dma_start(out=sin_sb[:], in_=sin2[:])
                nc.sync.dma_start(out=mask_sb[:], in_=mask[:])
                perm_f = cpool.tile([128, 128], f32)
                nc.sync.dma_start(out=perm_f[:], in_=perm[:])
                perm_r = cpool.tile([128, 128], f32r)
                nc.vector.tensor_copy(perm_r[:], perm_f[:])

                # head pair m: even head at partitions 0:64, odd at 64:128
                ctx_pair = ctxpool.tile([128, PAIRS, L_], f32r)

                # ================= Phase B: attention =================
                with (
                    tc.tile_pool(name="qkio", bufs=2) as iopool,
                    tc.tile_pool(name="qkr0", bufs=1) as r0pool,
                    tc.tile_pool(name="qkr", bufs=2) as qkrpool,
                    tc.tile_pool(name="ropetmp", bufs=2) as rtpool,
                    tc.tile_pool(name="pt", bufs=3) as ptpool,
                    tc.tile_pool(name="ctxsb", bufs=3) as cspool,
                    tc.tile_pool(name="norm", bufs=2) as nmpool,
                    tc.tile_pool(name="va", bufs=1) as vapool,
                    tc.tile_pool(name="psum_st", bufs=2, space="PSUM") as psST,
                    tc.tile_pool(name="psum_ctx", bufs=4, space="PSUM") as psCTX,
                ):
                    pend = []

                    def emit_norm(ctx_sb, hh, t, i0):
                        # 1/rowsum broadcast with no PE involvement: DMA the
                        # rowsum row to partition 0, DVE reciprocal, GPSIMD
                        # partition-broadcast, then normalize the context.
                        rsrow = nmpool.tile([1, 512], f32, name="rsrow")
                        nc.sync.dma_start(out=rsrow[:],
                                          in_=ctx_sb[64:65, :].bitcast(f32))
                        nc.vector.reciprocal(rsrow[:], rsrow[:])
                        rec = nmpool.tile([64, 512], f32, name="rec")
                        nc.gpsimd.partition_broadcast(rec[:], rsrow[:])
                        if hh == 0:
                            nc.vector.tensor_tensor(
                                ctx_pair[0:64, t, i0:i0 + 512],
                                ctx_sb[0:64, :], rec[:],
                                mybir.AluOpType.mult)
                        else:
                            # odd head lives at partitions 64:128 of ctx_pair;
                            # engines cannot shift partitions, so: f32 multiply
                            # -> sbuf-to-sbuf DMA shift -> fp32r rounding copy.
                            ctmp = nmpool.tile([64, 512], f32, name="ctmp")
                            nc.vector.tensor_tensor(
                                ctmp[:], ctx_sb[0:64, :], rec[:],
                                mybir.AluOpType.mult)
                            cst = nmpool.tile([128, 512], f32, name="cstage")
                            nc.sync.dma_start(out=cst[64:128, :], in_=ctmp[:])
                            nc.vector.tensor_copy(
                                ctx_pair[64:128, t, i0:i0 + 512],
                                cst[64:128, :])

                    for t in range(PAIRS):
                        qp_f = iopool.tile([128, L_], f32, name="qp_f")
                        kp_f = iopool.tile([128, L_], f32, name="kp_f")
                        nc.sync.dma_start(out=qp_f[:],
                                          in_=qk_s[128 * t:128 * t + 128, :])
                        nc.sync.dma_start(
                            out=kp_f[:],
                            in_=qk_s[QK + 128 * t:QK + 128 * t + 128, :])
                        qp_r0 = r0pool.tile([128, L_], f32r, name="qp_r0")
                        kp_r0 = r0pool.tile([128, L_], f32r, name="kp_r0")
                        nc.vector.tensor_copy(qp_r0[:], qp_f[:])
                        nc.vector.tensor_copy(kp_r0[:], kp_f[:])
                        qp_r = qkrpool.tile([128, L_], f32r, name="qp_r")
                        # k lhsT is zero-padded to a full K=128 contraction per
                        # head (rows of the other head = exact zeros); the q rhs
                        # needs no padding — its other-head rows are multiplied
                        # by those zeros. Full-K matmuls keep the PE HAM clock
                        # governor warm (partial row-group matmuls do not).
                        kpad = qkrpool.tile([128, 2, L_], f32r, name="kpad")
                        nc.vector.tensor_scalar_mul(kpad[64:128, 0, :],
                                                    kp_f[64:128, :], 0.0)
                        nc.vector.tensor_scalar_mul(kpad[0:64, 1, :],
                                                    kp_f[0:64, :], 0.0)
                        for lc in range(L_ // 512):
                            s = slice(512 * lc, 512 * lc + 512)
                            # q rope
                            rot = psST.tile([128, 1024], f32, name="st2")[:, :512]
                            nc.tensor.matmul(rot, perm_r[:], qp_r0[:, s],
                                             start=True, stop=True)
                            u1 = rtpool.tile([128, 512], f32, name="u1")
                            nc.vector.tensor_tensor(u1[:], rot, sin_sb[:, s],
                                                    mybir.AluOpType.mult)
                            u2 = rtpool.tile([128, 512], f32, name="u2")
                            nc.gpsimd.tensor_tensor(u2[:], qp_f[:, s],
                                                    cos_sb[:, s],
                                                    mybir.AluOpType.mult)
                            nc.vector.tensor_tensor(qp_r[:, s], u1[:], u2[:],
                                                    mybir.AluOpType.add)
                            # k rope, written into the padded per-head slots
                            rot2 = psST.tile([128, 1024], f32, name="st2")[:, :512]
                            nc.tensor.matmul(rot2, perm_r[:], kp_r0[:, s],
                                             start=True, stop=True)
                            u3 = rtpool.tile([128, 512], f32, name="u1")
                            nc.vector.tensor_tensor(u3[:], rot2, sin_sb[:, s],
                                                    mybir.AluOpType.mult)
                            u4 = rtpool.tile([128, 512], f32, name="u2")
                            nc.gpsimd.tensor_tensor(u4[:], kp_f[:, s],
                                                    cos_sb[:, s],
                                                    mybir.AluOpType.mult)
                            nc.vector.tensor_tensor(kpad[0:64, 0, s],
                                                    u3[0:64, :], u4[0:64, :],
                                                    mybir.AluOpType.add)
                            nc.vector.tensor_tensor(kpad[64:128, 1, s],
                                                    u3[64:128, :], u4[64:128, :],
                                                    mybir.AluOpType.add)

                        # v (with ones cols) for both heads of the pair
                        va = vapool.tile([128, LT, 2 * (HD + 1)], f32r,
                                         name="va")
                        vst = vapool.tile([128, LT, 2 * (HD + 1)], f32,
                                          name="vst")
                        nc.sync.dma_start(
                            out=vst[:],
                            in_=v_s[:, 2 * t:2 * t + 2, :].rearrange(
                                "(jt p) h d -> p jt (h d)", p=128))
                        nc.vector.tensor_copy(va[:], vst[:])

                        for ic in range(ICS):
                            i0 = 512 * ic
                            njt = 4 * ic + 4
                            ctx_ps = [psCTX.tile([65, 512], f32, name="ctx_ps")
                                      for _ in range(2)]
                            pend_new = []
                            for jg in range(njt // 2):
                                st2 = [psST.tile([128, 1024], f32, name="st2")
                                       for _ in range(2)]
                                # interleave u-inner so adjacent matmuls hit
                                # disjoint PE row groups and run concurrently
                                for u in range(2):
                                    jt = 2 * jg + u
                                    for hh in range(2):
                                        nc.tensor.matmul(
                                            st2[hh][:, 512 * u:512 * u + 512],
                                            kpad[:, hh, 128 * jt:128 * jt + 128],
                                            qp_r[:, i0:i0 + 512],
                                            start=True, stop=True)
                                for u in range(2):
                                    jt = 2 * jg + u
                                    o = 128 * jt - i0
                                    if o >= 0:  # diagonal stripe -> mask
                                        w = min(512, o + 128)
                                        for hh in range(2):
                                            nc.vector.tensor_tensor(
                                                st2[hh][:, 512 * u:512 * u + w],
                                                st2[hh][:, 512 * u:512 * u + w],
                                                mask_sb[:, 384 - o:384 - o + w],
                                                mybir.AluOpType.add)
                                pt2 = [ptpool.tile([128, 1024], f32r, name="pt2")
                                       for _ in range(2)]
                                for hh in range(2):
                                    nc.scalar.activation(
                                        pt2[hh][:], st2[hh][:],
                                        mybir.ActivationFunctionType.Exp)
                                for u in range(2):
                                    jt = 2 * jg + u
                                    for hh in range(2):
                                        nc.tensor.matmul(
                                            ctx_ps[hh][:],
                                            va[:, jt, 65 * hh:65 * hh + 65],
                                            pt2[hh][:, 512 * u:512 * u + 512],
                                            start=(jt == 0), stop=(jt == njt - 1))
                            for hh in range(2):
                                ctx_sb = cspool.tile([65, 512], f32r,
                                                     name="ctx_sb")
                                nc.scalar.copy(ctx_sb[:], ctx_ps[hh][:])
                                pend_new.append((ctx_sb, hh, t, i0))
                            # emit the previous chunk's normalization now —
                            # one step behind, so these latency chains fill
                            # engine idle slots instead of blocking the next
                            # chunk's matmul-feeding work (in-order queues).
                            for p in pend:
                                emit_norm(*p)
                            pend = pend_new

                    for p in pend:
                        emit_norm(*p)

                # ================= Phase C: output projection =================
                with (
                    tc.tile_pool(name="wo", bufs=1) as wopool,
                    tc.tile_pool(name="outC", bufs=4) as ocpool,
                    tc.tile_pool(name="psumC", bufs=6, space="PSUM") as psC,
                ):
                    wof = wopool.tile([128, PAIRS, D_], f32)
                    nc.sync.dma_start(out=wof[:], in_=woT[:])
                    wor = wopool.tile([128, PAIRS, D_], f32r)
                    nc.vector.tensor_copy(wor[:], wof[:])
                    for lt in range(LT):
                        for oc in range(D_ // 512):
                            ps = psC.tile([128, 512], f32, name="psC")
                            for m in range(PAIRS):
                                nc.tensor.matmul(
                                    ps[:], ctx_pair[:, m, 128 * lt:128 * lt + 128],
                                    wor[:, m, 512 * oc:512 * oc + 512],
                                    start=(m == 0), stop=(m == PAIRS - 1))
                            osb = ocpool.tile([128, 512], f32, name="osb")
                            nc.scalar.copy(osb[:], ps[:])
                            nc.sync.dma_start(
                                out=out[128 * lt:128 * lt + 128,
                                        512 * oc:512 * oc + 512],
                                in_=osb[:])

    nc.finalize()
    return nc


# ---------------------------------------------------------------- host prep
def _host_constants(L_=L):
    inv_freq = 1.0 / (LRPE_BASE ** (np.arange(0, HD, 2, dtype=np.float32) / HD))
    t = np.arange(L_, dtype=np.float32)
    freqs = t[:, None] * inv_freq[None, :]
    emb = np.concatenate([freqs, freqs], axis=-1)          # (L, 64)
    cosT = np.cos(emb).T.astype(np.float32)                # (64, L)
    sinT = np.sin(emb).T.astype(np.float32)
    cos2 = np.ascontiguousarray(np.tile(cosT, (2, 1)))     # (128, L)
    sin2 = np.ascontiguousarray(np.tile(sinT, (2, 1)))
    perm64 = np.zeros((64, 64), dtype=np.float32)
    for dp in range(32):
        perm64[dp + 32, dp] = -1.0
    for dp in range(32, 64):
        perm64[dp - 32, dp] = 1.0
    perm = np.zeros((128, 128), dtype=np.float32)
    perm[0:64, 0:64] = perm64
    perm[64:128, 64:128] = perm64
    maskbig = np.where(
        np.arange(896)[None, :] < np.arange(128)[:, None] + 384, NEG,
        np.float32(0.0)).astype(np.float32)
    return cos2, sin2, perm, maskbig


def _in_maps(x, wqkv, wo):
    cos2, sin2, perm, maskbig = _host_constants()
    scale = np.float32(HD ** -0.5)
    xT = [np.ascontiguousarray(x[b].T) for b in range(B)]
    maps = []
    for c in range(N_CORES):
        b, g = c // GROUPS, c % GROUPS
        r0 = HD * HPG * g
        wq = wqkv[r0:r0 + HD * HPG] * scale
        wk = wqkv[D + r0:D + r0 + HD * HPG]
        wv = wqkv[2 * D + r0:2 * D + r0 + HD * HPG]
        # wo block columns for this group, pair-stacked: (128, PAIRS, D)
        wo_blk = wo[:, HD * HPG * g:HD * HPG * (g + 1)].T  # (512, D), row=64h+d
        wo_pair = np.ascontiguousarray(
            wo_blk.reshape(HPG // 2, 128, D).transpose(1, 0, 2))
        maps.append({
            "xT": xT[b],
            "wqkT": np.ascontiguousarray(np.concatenate([wq, wk], 0).T),
            "wvT": np.ascontiguousarray(wv.T),
            "woT": wo_pair,
            "cos2": cos2, "sin2": sin2, "perm": perm, "mask": maskbig,
        })
    return maps


def run(x, wqkv, wo, trace=False):
    _env_setup()
    from concourse.bass_utils import run_bass_kernel_spmd

    if "prog" not in _PROGRAM_CACHE:
        _PROGRAM_CACHE["prog"] = build_program()
    nc = _PROGRAM_CACHE["prog"]
    maps = _in_maps(np.asarray(x), np.asarray(wqkv), np.asarray(wo))
    res = run_bass_kernel_spmd(nc, maps, list(range(N_CORES)), trace=trace)
    out = np.zeros((B, L, D), dtype=np.float32)
    for c in range(N_CORES):
        out[c // GROUPS] += res.results[c]["out"]
    return out, res


def kernel(x, wqkv, wo):
    out, _ = run(x, wqkv, wo, trace=False)
    return out
